# revision 1
# baseline (speedup 1.0000x reference)
"""Bass/Tile TRN2 kernel for nn_MultiHeadAttention (B=2, T=2048, C=1024, H=16, D=64).

Sharding (8 cores): core c -> batch b = c // 4, heads [4*(c%4) .. 4*(c%4)+3]
(tensor-parallel on heads x data-parallel on batch). Each core computes its
4 heads' attention plus its slice of the output projection (rows of Wp for
its heads), producing a partial [T, C]; the host sums the 4 partials per
batch (the "all-reduce" is done host-side since the full output is gathered
host-side anyway).

Per-core dataflow (all matmul operands float32r => full PE rate):
  xT [C, T] (pre-transposed on host)
  - Q.T, K.T per head-pair: psum[128, 512] = sum_k Wq2[k].T @ xT[k]  (2 heads
    packed on the partition axis; k-outer loop amortizes LDWEIGHTS over 4 MMs)
  - V: psum[t, 256] = sum_k xT[k][:, t128].T @ Wv4[k]  (4 heads packed)
  - scores S.T[tk] = K.T[tk-slice].T @ Q.T[qc-slice]   (2 heads packed in PE
    row groups via tile_position)
  - P = exp(0.125 * S.T) on ACT (PSUM -> SBUF fp32r), causal staircase mask
    on DVE for diagonal tiles, masked column ranges skipped everywhere
  - O.T[h] += [V|1][tk].T @ P.T[tk]  (fused row-sums into PSUM row 64)
  - normalize O.T by 1/rowsum: reciprocal_approx on DVE; the row broadcast
    across partitions is a K=1 matmul against constant head-selector rows
  - partial[t, c] = sum_pair OT_norm[pair][:, t128].T @ Wp[pair-rows, c512],
    interleaved with the attention of the next Tq chunk to keep PE dense
"""

import numpy as np

B, T, C = 2, 2048, 1024
H = 16
D = C // H  # 64
N_CORES = 8
PAIRS = 2  # head-pairs per core
KC = C // 128  # 8 contraction chunks
TT = T // 128  # 16 T tiles
QC = T // 512  # 4 Tq chunks

_CACHE = {}


def _build():
    import concourse.mybir as mybir
    import concourse.tile as tile
    from concourse import bacc

    f32 = mybir.dt.float32
    f32r = mybir.dt.float32r

    nc = bacc.Bacc("TRN2", target_bir_lowering=False, debug=False,
                   num_devices=N_CORES)

    xT_d = nc.dram_tensor("xT", [C, T], f32r, kind="ExternalInput").ap()
    wq_d = nc.dram_tensor("wq", [PAIRS, C, 128], f32r, kind="ExternalInput").ap()
    wk_d = nc.dram_tensor("wk", [PAIRS, C, 128], f32r, kind="ExternalInput").ap()
    wv_d = nc.dram_tensor("wv", [C, 256], f32r, kind="ExternalInput").ap()
    wp_d = nc.dram_tensor("wp", [256, C], f32r, kind="ExternalInput").ap()
    ones_d = nc.dram_tensor("ones", [128, 1], f32r, kind="ExternalInput").ap()
    hsel_d = nc.dram_tensor("hsel", [2, 128], f32r, kind="ExternalInput").ap()
    out_d = nc.dram_tensor("out", [T, C], f32, kind="ExternalOutput").ap()

    with tile.TileContext(nc) as tc:
        _emit(nc, tc, tile, mybir, xT_d, wq_d, wk_d, wv_d, wp_d, ones_d,
              hsel_d, out_d)

    nc.compile()
    return nc


def _emit(nc, tc, tile, mybir, xT_d, wq_d, wk_d, wv_d, wp_d, ones_d, hsel_d,
          out_d, dbg=None):
    from contextlib import ExitStack

    f32 = mybir.dt.float32
    f32r = mybir.dt.float32r

    ctx = ExitStack()
    with ctx:
        # ---- long-lived pools ----
        qt_pool = ctx.enter_context(tc.tile_pool(name="qt", bufs=1))
        v_pool = ctx.enter_context(tc.tile_pool(name="v", bufs=1))
        ot_pool = ctx.enter_context(tc.tile_pool(name="ot", bufs=1))
        const_pool = ctx.enter_context(tc.tile_pool(name="const", bufs=1))
        wp_pool = ctx.enter_context(tc.tile_pool(name="wp", bufs=1))
        p_pool = ctx.enter_context(tc.tile_pool(name="p", bufs=5))
        misc_pool = ctx.enter_context(tc.tile_pool(name="misc", bufs=2))
        ostage_pool = ctx.enter_context(tc.tile_pool(name="ostage", bufs=2))
        psum_pool = ctx.enter_context(tc.tile_pool(name="psum", bufs=2,
                                                   space="PSUM"))

        ones_sb = const_pool.tile([128, 1], f32r, tag="ones", name="ones_sb")
        nc.sync.dma_start(ones_sb[:], ones_d[:])
        hsel_sb = [const_pool.tile([1, 128], f32r, tag=f"hsel{h}",
                                   name=f"hsel_sb{h}") for h in range(2)]
        for h in range(2):
            nc.sync.dma_start(hsel_sb[h][:], hsel_d[h:h + 1, :])

        # universal causal staircase, doubled for the 2 packed heads:
        # stair[kk, a, qq] = 1.0 if qq - kk >= 0 else 0.0
        stair = const_pool.tile([128, 2, 128], f32, tag="stair", name="stair")
        nc.gpsimd.memset(stair[:], 1.0)
        nc.gpsimd.affine_select(
            out=stair[:], in_=stair[:],
            compare_op=mybir.AluOpType.is_ge,
            fill=0.0,
            base=0,
            pattern=[[0, 2], [1, 128]],
            channel_multiplier=-1,
        )

        wp_sb = []
        for p in range(PAIRS):
            w = wp_pool.tile([128, C], f32r, tag=f"wp{p}", name=f"wpsb{p}")
            nc.sync.dma_start(w[:], wp_d[128 * p:128 * (p + 1), :])
            wp_sb.append(w)

        qt_sb = [qt_pool.tile([128, T], f32r, tag=f"qt{p}", name=f"qt{p}")
                 for p in range(PAIRS)]
        kt_sb = [qt_pool.tile([128, T], f32r, tag=f"kt{p}", name=f"kt{p}")
                 for p in range(PAIRS)]
        # v_sb[p][h]: [128, TT*65]; Tk tile tk = cols [65tk, 65tk+64) plus a
        # ones column at 65tk+64 (feeds the fused row-sum PSUM row)
        v_sb = [[v_pool.tile([128, TT * 65], f32r, tag=f"v{p}{h}",
                             name=f"v{p}{h}") for h in range(2)]
                for p in range(PAIRS)]
        for p in range(PAIRS):
            for h in range(2):
                v3 = v_sb[p][h][:].rearrange("p (t c) -> p t c", c=65)
                nc.vector.tensor_copy(
                    v3[:, :, 64:65],
                    ones_sb[:].unsqueeze(1).broadcast_to([128, TT, 1]),
                )
        ot_sb = [ot_pool.tile([128, T], f32r, tag=f"ot{p}", name=f"ot{p}")
                 for p in range(PAIRS)]

        # ---- phase 1: projections (xT + weights resident) ----
        with tc.tile_pool(name="xt", bufs=1) as xt_pool, \
             tc.tile_pool(name="w", bufs=1) as w_pool:
            wv_sb = w_pool.tile([128, KC * 256], f32r, tag="wv")
            wq_sb, wk_sb = [], []
            for p in range(PAIRS):
                wq_ = w_pool.tile([128, KC * 128], f32r, tag="wq", bufs=2,
                                  name=f"wq_{p}")
                wk_ = w_pool.tile([128, KC * 128], f32r, tag="wk", bufs=2,
                                  name=f"wk_{p}")
                nc.sync.dma_start(
                    wq_[:].rearrange("p (k c) -> p k c", k=KC),
                    wq_d[p].rearrange("(k p) c -> p k c", p=128))
                nc.sync.dma_start(
                    wk_[:].rearrange("p (k c) -> p k c", k=KC),
                    wk_d[p].rearrange("(k p) c -> p k c", p=128))
                wq_sb.append(wq_)
                wk_sb.append(wk_)
            nc.sync.dma_start(
                wv_sb[:].rearrange("p (k c) -> p k c", k=KC),
                wv_d[:].rearrange("(k p) c -> p k c", p=128))
            xt = []
            for k in range(KC):
                t_ = xt_pool.tile([128, T], f32r, tag=f"xt{k}", name=f"xtsb{k}")
                nc.sync.dma_start(t_[:], xT_d[128 * k:128 * (k + 1), :])
                xt.append(t_)

            for p in range(PAIRS):
                wq_, wk_ = wq_sb[p], wk_sb[p]
                # Q.T / K.T: k-outer so each weight chunk's LDWEIGHTS covers
                # 4 N=512 matmuls into 4 psum banks (2 x [128,1024] tiles)
                for (w_, dst) in ((wq_, qt_sb[p]), (wk_, kt_sb[p])):
                    ps01 = psum_pool.tile([128, 1024], f32, tag="psA",
                                          name="proj01", bufs=2)
                    ps23 = psum_pool.tile([128, 1024], f32, tag="psA",
                                          name="proj23", bufs=2)
                    for k in range(KC):
                        for q4 in range(QC):
                            tgt = (ps01, ps23)[q4 // 2]
                            nc.tensor.matmul(
                                tgt[:, 512 * (q4 % 2):512 * (q4 % 2 + 1)],
                                w_[:, 128 * k:128 * (k + 1)],
                                xt[k][:, 512 * q4:512 * (q4 + 1)],
                                start=(k == 0), stop=(k == KC - 1),
                                skip_group_check=True,
                            )
                    for q4 in range(QC):
                        tgt = (ps01, ps23)[q4 // 2]
                        nc.vector.tensor_copy(
                            dst[:, 512 * q4:512 * (q4 + 1)],
                            tgt[:, 512 * (q4 % 2):512 * (q4 % 2 + 1)])

                # V: per T-tile [128, 256] (4 heads), accumulated over k;
                # double-buffered by alternating the psB/psC bank tags
                if p == 0:
                    for t_ in range(TT):
                        ps = psum_pool.tile([128, 256], f32,
                                            tag=("psB", "psC")[t_ % 2],
                                            name="v_ps", bufs=2)
                        for k in range(KC):
                            nc.tensor.matmul(
                                ps[:],
                                xt[k][:, 128 * t_:128 * (t_ + 1)],
                                wv_sb[:, 256 * k:256 * (k + 1)],
                                start=(k == 0), stop=(k == KC - 1),
                            )
                        for pp in range(PAIRS):
                            for h in range(2):
                                nc.vector.tensor_copy(
                                    v_sb[pp][h][:, 65 * t_:65 * t_ + 64],
                                    ps[:, 128 * pp + 64 * h:
                                       128 * pp + 64 * (h + 1)],
                                )

        # ---- phase 2+3: attention (qc-outer, pairs interleaved per tk to
        # keep the PE dense) with the output projection of finished Tq
        # chunks interleaved as filler ----
        for qc in range(QC):
            n_tk = 4 * (qc + 1)
            ot_ps_all = [[psum_pool.tile([65, 512], f32,
                                         tag=("psB", "psC")[pp], bufs=2,
                                         name=f"ot_ps{pp}{h}")
                          for h in range(2)] for pp in range(PAIRS)]
            for tk in range(n_tk):
                for p in range(PAIRS):
                    ot_ps = ot_ps_all[p]
                    s_ps = psum_pool.tile([128, 1024], f32, tag="psA",
                                          name="s_ps", bufs=2)
                    for h in range(2):
                        nc.tensor.matmul(
                            s_ps[:, 512 * h:512 * (h + 1)],
                            kt_sb[p][64 * h:64 * (h + 1),
                                     128 * tk:128 * (tk + 1)],
                            qt_sb[p][64 * h:64 * (h + 1),
                                     512 * qc:512 * (qc + 1)],
                            start=True, stop=True,
                            tile_position=(64 * h, 0),
                        )
                    # exp on ACT -> SBUF fp32r (restricted to unmasked cols)
                    j = tk - 4 * qc  # diagonal sub-position on diag tiles
                    lo = 128 * j if j > 0 else 0
                    p_sb = p_pool.tile([128, 1024], f32r, tag="p_sb")
                    s3 = s_ps[:].rearrange("p (a q) -> p a q", a=2)
                    p3 = p_sb[:].rearrange("p (a q) -> p a q", a=2)
                    nc.scalar.activation(p3[:, :, lo:512], s3[:, :, lo:512],
                                         mybir.ActivationFunctionType.Exp,
                                         scale=0.125)
                    if j >= 0:
                        nc.vector.tensor_mul(p3[:, :, lo:lo + 128],
                                             p3[:, :, lo:lo + 128],
                                             stair[:])
                    first, last = (tk == 0), (tk == n_tk - 1)
                    for h in range(2):
                        nc.tensor.matmul(
                            ot_ps[h][:, lo:512],
                            v_sb[p][h][:, 65 * tk:65 * (tk + 1)],
                            p_sb[:, 512 * h + lo:512 * (h + 1)],
                            start=first, stop=last,
                        )
            for p in range(PAIRS):
                ot_ps = ot_ps_all[p]
                # normalize: OT_norm[:, qc] = O.T * (1/rowsum); rowsum recip
                # broadcast across partitions via K=1 matmul on selectors
                bcast_ps = psum_pool.tile([128, 512], f32, tag="psA",
                                          name="bcast_ps", bufs=2)
                for h in range(2):
                    recip_sb = misc_pool.tile([65, 512], f32, tag="recip",
                                              name="recip_sb")
                    nc.vector.reciprocal_approx_fast(recip_sb[:, :],
                                                     ot_ps[h][:, :])
                    recip_row = misc_pool.tile([1, 512], f32r, tag="recip_row",
                                               name="recip_row")
                    nc.vector.tensor_copy(recip_row[:], recip_sb[64:65, :])
                    nc.tensor.matmul(bcast_ps[:], hsel_sb[h][:],
                                     recip_row[:], start=(h == 0),
                                     stop=(h == 1))
                bcast_sb = misc_pool.tile([128, 512], f32, tag="bcast",
                                          name="bcast_sb")
                nc.vector.tensor_copy(bcast_sb[:], bcast_ps[:])
                for h in range(2):
                    nc.vector.tensor_mul(
                        ot_sb[p][64 * h:64 * (h + 1),
                                 512 * qc:512 * (qc + 1)],
                        ot_ps[h][0:64, :],
                        bcast_sb[64 * h:64 * (h + 1), :])

            # output projection for this finished Tq chunk (4 T-tiles)
            for t_ in range(4 * qc, 4 * qc + 4):
                out_ps = [psum_pool.tile([128, 512], f32,
                                         tag=("psB", "psC")[c2], bufs=2,
                                         name=f"o_ps{c2}")
                          for c2 in range(2)]
                for p2 in range(PAIRS):
                    for c2 in range(2):
                        nc.tensor.matmul(
                            out_ps[c2][:],
                            ot_sb[p2][:, 128 * t_:128 * (t_ + 1)],
                            wp_sb[p2][:, 512 * c2:512 * (c2 + 1)],
                            start=(p2 == 0), stop=(p2 == PAIRS - 1),
                        )
                stage = ostage_pool.tile([128, C], f32, tag="stage",
                                         name="stage")
                for c2 in range(2):
                    nc.any.tensor_copy(stage[:, 512 * c2:512 * (c2 + 1)],
                                       out_ps[c2][:])
                nc.sync.dma_start(out_d[128 * t_:128 * (t_ + 1), :], stage[:])


def _get_nc():
    if "nc" not in _CACHE:
        _CACHE["nc"] = _build()
    return _CACHE["nc"]


def make_in_maps(x, Wq, Wk, Wv, Wp):
    x = np.asarray(x, dtype=np.float32)
    Wq = np.asarray(Wq, dtype=np.float32)
    Wk = np.asarray(Wk, dtype=np.float32)
    Wv = np.asarray(Wv, dtype=np.float32)
    Wp = np.asarray(Wp, dtype=np.float32)

    ones = np.ones((128, 1), dtype=np.float32)
    hsel = np.zeros((2, 128), dtype=np.float32)
    hsel[0, 0:64] = 1.0
    hsel[1, 64:128] = 1.0
    in_maps = []
    for c in range(N_CORES):
        b = c // 4
        h0 = 4 * (c % 4)  # first of the 4 heads on this core
        hs = list(range(h0, h0 + 4))
        xT = np.ascontiguousarray(x[b].T)  # [C, T]
        wq = np.stack([np.concatenate([Wq[hs[2 * p]], Wq[hs[2 * p + 1]]],
                                      axis=1) for p in range(PAIRS)])
        wk = np.stack([np.concatenate([Wk[hs[2 * p]], Wk[hs[2 * p + 1]]],
                                      axis=1) for p in range(PAIRS)])
        wv = np.concatenate([Wv[h] for h in hs], axis=1)  # [C, 256]
        wp = Wp[D * h0:D * (h0 + 4), :]  # [256, C]
        in_maps.append({"xT": xT, "wq": wq, "wk": wk, "wv": wv, "wp": wp,
                        "ones": ones, "hsel": hsel})
    return in_maps


def kernel(x, Wq, Wk, Wv, Wp):
    from concourse.bass_utils import run_bass_kernel_spmd

    in_maps = make_in_maps(x, Wq, Wk, Wv, Wp)
    nc = _get_nc()
    res = run_bass_kernel_spmd(nc, in_maps, list(range(N_CORES)))

    out = np.zeros((B, T, C), dtype=np.float32)
    for c in range(N_CORES):
        out[c // 4] += res.results[c]["out"]
    return out

